# revision 12
# baseline (speedup 1.0000x reference)
"""Decorrelation (ZCA-whitening) normalization kernel for Trainium2 (Bass/Tile).

Full input (64, 56, 56, 256) f32. Data-parallel over batch across 8 NeuronCores
(8 batches -> 25088 pixels per core). Per core:

  Pass 1: stream pixel-major chunks from HBM with 2KB DMA segments (row-pair
          layout: partition p holds DRAM rows 2p,2p+1 of a 256-row block),
          cast to fp16 (gpsimd), accumulate per-half 128x129 Gram blocks on
          the PE (extra ones-column yields channel sums for free), PE-transpose
          every (128px,128ch) tile to channel-major fp16 resident in SBUF
          (copies split across vector/scalar).
  Stats:  compact the block-diagonal Gram to a (34,128) payload (PE select +
          transpose matmuls), one 17KB AllReduce, expand back on each core.
  NS:     trace-normalized Newton-Schulz in Q-form (scale factors folded into
          per-iteration constants): per iter AB = Q@[Q|Sn] (one 256-wide
          matmul), C = A@B, Q' = s_k*C + Q. Both halves interleaved.
  Pass 2: whiten matmul from the fp16 resident tiles (lhsT=res slice,
          rhs=wm16), mean subtraction folded in as a K=1 rank-1 matmul
          accumulating -wm@mu into the same PSUM group, staging copies split
          vector/scalar, 2KB-segment output DMA.

HBM traffic per core = 1x read + 1x write.
"""

import sys

import numpy as np

for _p in ("/root/.axon_site/_ro/trn_rl_repo", "/opt/trn_rl_repo"):
    if _p not in sys.path:
        sys.path.append(_p)

# ---------------------------------------------------------------- constants
B, W, H, C = 64, 56, 56, 256
N_CORES = 8
B_LOC = B // N_CORES                # 8 batches per core
N_LOC = B_LOC * W * H               # 25088 pixels per core
N_TOT = B * W * H                   # 200704 pixels total
P = 128                             # partitions
J = 7                               # row-pair groups per chunk
T = 2                               # rows per pair
CPX = J * T * P                     # 1792 pixels per chunk
NCHUNK = N_LOC // CPX               # 14 chunks per core
EPS = 1e-3
ITER_NUM = 5

assert NCHUNK * CPX == N_LOC

_STATE = {}


def _build_nc(variant=()):
    import concourse.bacc as bacc
    import concourse.tile as tile
    from concourse import mybir
    from contextlib import ExitStack

    f32 = mybir.dt.float32
    f16 = mybir.dt.float16
    Alu = mybir.AluOpType
    Act = mybir.ActivationFunctionType
    Axis = mybir.AxisListType

    nc = bacc.Bacc("TRN2", target_bir_lowering=False, debug=False,
                   num_devices=N_CORES)

    x = nc.dram_tensor("x", [N_LOC, C], f32, kind="ExternalInput").ap()
    y = nc.dram_tensor("y", [N_LOC, C], f32, kind="ExternalOutput").ap()
    c_id16 = nc.dram_tensor("c_id16", [P, P], f16, kind="ExternalInput").ap()
    c_eye = nc.dram_tensor("c_eye", [P, P], f32, kind="ExternalInput").ap()
    c_epseye = nc.dram_tensor("c_epseye", [P, P], f32, kind="ExternalInput").ap()
    c_mask = nc.dram_tensor("c_mask", [P, P], f32, kind="ExternalInput").ap()
    c_bsel = nc.dram_tensor("c_bsel", [P, 16], f32, kind="ExternalInput").ap()
    c_brep = nc.dram_tensor("c_brep", [64, P], f32, kind="ExternalInput").ap()
    c_ones16 = nc.dram_tensor("c_ones16", [1, P], f16, kind="ExternalInput").ap()
    d_ccsb = nc.dram_tensor("d_ccsb", [33, 2, P], f32, kind="ExternalOutput").ap()
    d_arst = nc.dram_tensor("d_arst", [33, 2, P], f32, kind="ExternalOutput").ap()
    d_wm0 = nc.dram_tensor("d_wm0", [P, P], f32, kind="ExternalOutput").ap()
    d_wm1 = nc.dram_tensor("d_wm1", [P, P], f32, kind="ExternalOutput").ap()
    d_nwm = nc.dram_tensor("d_nwm", [2, 4, P], f32, kind="ExternalOutput").ap()

    # Q-form NS constants: Q1 = I - Sn/3; Q_{k+1} = s_k*C_k + Q_k with
    # C_k = Q^3 Sn; final wm = f5 * Q5 * rsqrt(trace).
    fks = [1.5]
    for _ in range(ITER_NUM - 1):
        fks.append(fks[-1] * 1.5)
    sks = [-(fks[k] ** 2) / 3.0 for k in range(ITER_NUM - 1)]
    f_fin = fks[-1]

    with tile.TileContext(nc) as tc, ExitStack() as octx:
        # ---------------- long-lived pools
        consts = octx.enter_context(tc.tile_pool(name="consts", bufs=1))
        resp = octx.enter_context(tc.tile_pool(name="resident", bufs=1))
        statp = octx.enter_context(tc.tile_pool(name="stats", bufs=1))
        gps = octx.enter_context(tc.tile_pool(name="gpsum", bufs=1, space="PSUM"))

        id16 = consts.tile([P, P], f16, name="id16")
        eye = consts.tile([P, P], f32, name="eye")
        epseye = consts.tile([P, P], f32, name="epseye")
        mask = consts.tile([P, P], f32, name="mask")
        bsel = consts.tile([P, 16], f32, name="bsel")
        brep = consts.tile([64, P], f32, name="brep")
        ones16 = consts.tile([1, P], f16, name="ones16")
        nc.sync.dma_start(out=id16, in_=c_id16)
        nc.sync.dma_start(out=eye, in_=c_eye)
        nc.sync.dma_start(out=epseye, in_=c_epseye)
        nc.sync.dma_start(out=mask, in_=c_mask)
        nc.sync.dma_start(out=bsel, in_=c_bsel)
        nc.sync.dma_start(out=brep, in_=c_brep)
        nc.sync.dma_start(out=ones16, in_=c_ones16)

        # channel-major fp16 resident tiles: one per (chunk, half)
        res = [[resp.tile([P, J, T, P], f16, name=f"res_{c}_{h}")
                for h in range(2)] for c in range(NCHUNK)]

        # Gram PSUM: per half (128 x 129), col 128 accumulates channel sums
        g_ps = [gps.tile([P, 129], f32, name=f"G_{h}") for h in range(2)]

        # fp16 cast buffers with baked-in ones columns (manual ping-pong so
        # the ones columns are written exactly once per buffer)
        xh_bufs = [resp.tile([P, J, T, 2, 129], f16, name=f"xh_{k}")
                   for k in range(2)]
        for k in range(2):
            for h in range(2):
                nc.gpsimd.memset(xh_bufs[k][:, :, :, h, 128:129], 1.0)

        xv = x.rearrange("(c j p t) (hh ch) -> c p j t hh ch",
                         j=J, p=P, t=T, hh=2, ch=P)
        yv = y.rearrange("(c j p t) (hh ch) -> c p j t hh ch",
                         j=J, p=P, t=T, hh=2, ch=P)

        nrep = 1
        for v in variant:
            if v.startswith("rep"):
                nrep = int(v[3:])
        for _rep in range(nrep):
         # ================= PASS 1 =================
         with ExitStack() as ctx:
             loadp = ctx.enter_context(tc.tile_pool(name="loadp", bufs=2))
             trps = ctx.enter_context(tc.tile_pool(name="trpsum", bufs=4, space="PSUM"))

             cp_i = 0
             for ci in range(NCHUNK):
                 xt = loadp.tile([P, J, T, 2, P], f32, name="xt")
                 nc.sync.dma_start(out=xt, in_=xv[ci])
                 xh = xh_bufs[ci % 2]
                 # cast f32 -> fp16 on gpsimd (SBUF->SBUF; gpsimd has no
                 # PSUM port so it gets the cast, V/S get the PSUM copies)
                 for h in range(2):
                     nc.gpsimd.tensor_copy(out=xh[:, :, :, h, 0:128],
                                           in_=xt[:, :, :, h, :])

                 # Gram accumulation (fp16 in, f32 PSUM): G_h += sl^T @ [sl|1]
                 for j in range(J):
                     for t in range(T):
                         first = ci == 0 and j == 0 and t == 0
                         last = ci == NCHUNK - 1 and j == J - 1 and t == T - 1
                         if "nogram" in variant:
                             continue
                         for h in range(2):
                             sl = xh[:, j, t, h, 0:128]
                             rhs = xh[:, j, t, h, 0:129]
                             nc.tensor.matmul(g_ps[h], sl, rhs, start=first,
                                              stop=last, skip_group_check=True)

                 # PE transpose each (128px,128ch) tile -> channel-major fp16
                 for h in range(2 if "notr" not in variant else 0):
                     for b0 in range(0, J * T, 4):
                         bn = min(4, J * T - b0)
                         tp = trps.tile([P, 4, P], f16, name="tp")
                         for k in range(bn):
                             j, t = divmod(b0 + k, T)
                             nc.tensor.matmul(
                                 tp[:, k, :], xh[:, j, t, h, 0:128],
                                 id16, is_transpose=True, skip_group_check=True)
                         j0, t0 = divmod(b0, T)
                         j1 = divmod(b0 + bn - 1, T)[0] + 1
                         dst = res[ci][h][:, j0:j1, :, :]
                         eng = nc.vector if cp_i % 2 == 0 else nc.scalar
                         if eng is nc.vector:
                             nc.vector.tensor_copy(out=dst, in_=tp[:, :bn, :])
                         else:
                             nc.scalar.copy(out=dst, in_=tp[:, :bn, :])
                         cp_i += 1

         # ================= STATS + ALL-REDUCE =================
         with ExitStack() as ctx:
             dramp = ctx.enter_context(tc.tile_pool(name="dram", bufs=1, space="DRAM"))
             nsp = ctx.enter_context(tc.tile_pool(name="nsp", bufs=6))
             nps = ctx.enter_context(tc.tile_pool(name="nspsum", bufs=2, space="PSUM"))

             # masked gram + sums, compacted to a (33, 2, 128) payload:
             # partitions [0:16) = per-half compact gram rows (PE transpose
             # must land at partition 0), partition 32 = channel-sums row
             # (engine partition bases must be 32-aligned)
             Gm = statp.tile([P, 2, 129], f32, name="Gm")
             compactS = statp.tile([P, 2, 17], f32, name="compactS")
             cp_ps = nps.tile([P, 32], f32, name="cp_ps", tag="nsps")
             pay_ps = nps.tile([17, 2, P], f32, name="pay_ps", tag="nsps")
             sums_ps = nps.tile([1, 2, P], f32, name="sums_ps", tag="nsc")
             cc_sb = statp.tile([33, 2, P], f32, name="cc_sb")
             for h in range(2):
                 nc.vector.tensor_tensor(out=Gm[:, h, 0:128],
                                         in0=g_ps[h][:, 0:128], in1=mask,
                                         op=Alu.mult)
                 nc.scalar.copy(out=compactS[:, h, 16:17],
                                in_=g_ps[h][:, 128:129])
                 nc.tensor.matmul(cp_ps[:, h * 16:(h + 1) * 16],
                                  Gm[:, h, 0:128], bsel,
                                  skip_group_check=True)
                 nc.scalar.copy(out=compactS[:, h, 0:16],
                                in_=cp_ps[:, h * 16:(h + 1) * 16])
                 nc.tensor.matmul(pay_ps[0:16, h, :],
                                  compactS[:, h, 0:16], eye,
                                  is_transpose=True, skip_group_check=True)
                 nc.tensor.matmul(sums_ps[0:1, h, :],
                                  compactS[:, h, 16:17], eye,
                                  is_transpose=True, skip_group_check=True)
             nc.scalar.copy(out=cc_sb[0:16, :, :], in_=pay_ps[0:16, :, :])
             nc.scalar.copy(out=cc_sb[32:33, :, :], in_=sums_ps)

             arst = statp.tile([33, 2, P], f32, name="arst")
             if "nocc" in variant:
                 nc.vector.tensor_scalar_mul(out=arst, in0=cc_sb,
                                             scalar1=float(N_CORES))
             else:
                 cc_in = dramp.tile([33, 2, P], f32, name="cc_in")
                 cc_out = dramp.tile([33, 2, P], f32, name="cc_out")
                 nc.sync.dma_start(out=cc_in, in_=cc_sb)
                 nc.gpsimd.collective_compute(
                     "AllReduce", mybir.AluOpType.add,
                     replica_groups=[list(range(N_CORES))],
                     ins=[cc_in.opt()], outs=[cc_out.opt()])
                 nc.sync.dma_start(out=arst, in_=cc_out)

             # ============= Newton-Schulz (halves interleaved) =============
             wm16 = [statp.tile([P, P], f16, name=f"wm16_{h}") for h in range(2)]
             negwm16 = [statp.tile([1, 4, P], f16, name=f"negwm16_{h}")
                        for h in range(2)]
             nsbuf = [[statp.tile([P, 256], f32, name=f"nsb_{h}_{k}")
                       for k in range(2)] for h in range(2)]

             murow = [arst[32:33, h, :] for h in range(2)]
             sig = [None, None]
             tvec = [None, None]
             rinv = [None, None]

             for h in range(2):
                 # expand compact gram to block-diagonal (128,128)
                 gx_ps = nps.tile([P, P], f32, name=f"gx_{h}", tag="nsps")
                 nc.tensor.matmul(gx_ps, arst[0:16, h, :], brep[0:16, :],
                                  skip_group_check=True)
                 # outer product of channel sums (K=1 matmul)
                 o_ps = nps.tile([P, P], f32, name=f"o_{h}", tag="nsps")
                 nc.tensor.matmul(o_ps, murow[h], murow[h],
                                  skip_group_check=True)
                 osc = nsp.tile([P, P], f32, name="osc", tag="nsbig")
                 nc.scalar.activation(
                     out=osc, in_=o_ps, func=Act.Identity,
                     scale=-(1.0 - EPS) / (float(N_TOT) * float(N_TOT)))
                 # sigma = mask * ((1-eps)/N * G - (1-eps) mu mu^T) + eps I
                 s_t = nsp.tile([P, P], f32, name=f"sig_{h}", tag="sig")
                 nc.vector.scalar_tensor_tensor(
                     out=s_t, in0=gx_ps, scalar=(1.0 - EPS) / float(N_TOT),
                     in1=osc, op0=Alu.mult, op1=Alu.add)
                 nc.vector.tensor_mul(out=s_t, in0=s_t, in1=mask)
                 nc.vector.tensor_add(out=s_t, in0=s_t, in1=epseye)
                 sig[h] = s_t

             for h in range(2):
                 # per-group trace, spread to rows via mask matmul
                 djunk = nsp.tile([P, P], f32, name="djunk", tag="nsbig")
                 dcol = nsp.tile([P, 1], f32, name="dcol", tag="nssmall")
                 nc.vector.tensor_mul(out=djunk, in0=sig[h], in1=eye)
                 nc.vector.reduce_sum(out=dcol, in_=djunk, axis=Axis.X)
                 tv_ps = nps.tile([P, 1], f32, name="tv_ps", tag="nsps")
                 nc.tensor.matmul(tv_ps, mask, dcol, skip_group_check=True)
                 t_t = nsp.tile([P, 1], f32, name=f"tvec_{h}", tag="nssmall")
                 nc.scalar.copy(out=t_t, in_=tv_ps)
                 r_t = nsp.tile([P, 1], f32, name=f"rinv_{h}", tag="nssmall")
                 nc.vector.reciprocal(out=r_t, in_=t_t)
                 tvec[h] = t_t
                 rinv[h] = r_t

             for h in range(2):
                 # Sn into right half of both ping-pong buffers
                 nc.vector.tensor_scalar_mul(out=nsbuf[h][0][:, 128:256],
                                             in0=sig[h], scalar1=rinv[h])
                 nc.scalar.copy(out=nsbuf[h][1][:, 128:256],
                                in_=nsbuf[h][0][:, 128:256])
                 # Q1 = I - Sn/3
                 nc.vector.scalar_tensor_tensor(
                     out=nsbuf[h][0][:, 0:128],
                     in0=nsbuf[h][0][:, 128:256], scalar=-1.0 / 3.0,
                     in1=eye, op0=Alu.mult, op1=Alu.add)

             for k in range(ITER_NUM - 1):
                 src, dst = k % 2, (k + 1) % 2
                 ab_ps = [None, None]
                 absb = [None, None]
                 c_ps = [None, None]
                 for h in range(2):
                     ab_ps[h] = nps.tile([P, 256], f32, name=f"ab_{h}",
                                         tag="nsab")
                     nc.tensor.matmul(ab_ps[h], nsbuf[h][src][:, 0:128],
                                      nsbuf[h][src], skip_group_check=True)
                 for h in range(2):
                     absb[h] = nsp.tile([P, 256], f32, name=f"absb_{h}",
                                        tag=f"nsabsb{h}")
                     nc.scalar.copy(out=absb[h], in_=ab_ps[h])
                 for h in range(2):
                     c_ps[h] = nps.tile([P, P], f32, name=f"c_{h}",
                                        tag="nsc")
                     nc.tensor.matmul(c_ps[h], absb[h][:, 0:128],
                                      absb[h][:, 128:256],
                                      skip_group_check=True)
                 for h in range(2):
                     nc.vector.scalar_tensor_tensor(
                         out=nsbuf[h][dst][:, 0:128], in0=c_ps[h],
                         scalar=sks[k], in1=nsbuf[h][src][:, 0:128],
                         op0=Alu.mult, op1=Alu.add)

             if "dbg" in variant:
                 nc.sync.dma_start(out=d_ccsb, in_=cc_sb)
                 nc.sync.dma_start(out=d_arst, in_=arst)

             qfin = (ITER_NUM - 1) % 2
             for h in range(2):
                 # wm = f_fin * Q5 * rsqrt(trace)
                 sq = nsp.tile([P, 1], f32, name="sq", tag="nssmall")
                 nc.scalar.activation(out=sq, in_=tvec[h], func=Act.Sqrt)
                 rs = nsp.tile([P, 1], f32, name="rs", tag="nssmall")
                 nc.vector.reciprocal(out=rs, in_=sq)
                 wmf = nsp.tile([P, P], f32, name=f"wmf_{h}", tag="wmf")
                 nc.vector.tensor_scalar(out=wmf, in0=nsbuf[h][qfin][:, 0:128],
                                         scalar1=rs, scalar2=float(f_fin),
                                         op0=Alu.mult, op1=Alu.mult)
                 nc.vector.tensor_copy(out=wm16[h], in_=wmf)

                 # bias row: -wm @ mu, replicated 4x along free dim (fp16)
                 mu_ps = nps.tile([P, 1], f32, name="mu_ps", tag="nsps")
                 nc.tensor.matmul(mu_ps, murow[h], eye[32:33, 32:33],
                                  is_transpose=True, skip_group_check=True)
                 mucol = nsp.tile([P, 1], f32, name="mucol", tag="nssmall")
                 nc.scalar.activation(out=mucol, in_=mu_ps, func=Act.Identity,
                                      scale=1.0 / float(N_TOT))
                 nwm_ps = nps.tile([1, P], f32, name="nwm_ps", tag="nsps")
                 nc.tensor.matmul(nwm_ps, mucol, wmf, skip_group_check=True)
                 for q in range(4):
                     nc.scalar.activation(out=negwm16[h][:, q, :], in_=nwm_ps,
                                          func=Act.Identity, scale=-1.0)

         if "dbg" in variant:
             dbg_wm = [d_wm0, d_wm1]
             for h in range(2):
                 wmf32 = statp.tile([P, P], f32, name=f"dbgwm_{h}")
                 nc.vector.tensor_copy(out=wmf32, in_=wm16[h])
                 nc.sync.dma_start(out=dbg_wm[h], in_=wmf32)
                 nwf32 = statp.tile([1, 4, P], f32, name=f"dbgnw_{h}")
                 nc.vector.tensor_copy(out=nwf32, in_=negwm16[h])
                 nc.sync.dma_start(out=d_nwm[h], in_=nwf32)

         # ================= PASS 2 =================
         with ExitStack() as ctx:
             stagep = ctx.enter_context(tc.tile_pool(name="stagep", bufs=2))
             yps = ctx.enter_context(tc.tile_pool(name="ypsum", bufs=4, space="PSUM"))

             cp_i = 0
             for ci in range(NCHUNK if "nop2" not in variant else 0):
                 st = stagep.tile([P, J, T, 2, P], f32, name="st")
                 for h in range(2):
                     for b0 in range(0, J * T, 4):
                         bn = min(4, J * T - b0)
                         yp = yps.tile([P, 4, P], f32, name="yp")
                         # mean subtraction: rank-1 -wm@mu seeds the PSUM
                         # group, whiten matmuls accumulate on top
                         nc.tensor.matmul(yp[:, 0:bn, :], ones16,
                                          negwm16[h][:, 0:bn, :],
                                          start=True, stop=False,
                                          skip_group_check=True)
                         for k in range(bn):
                             j, t = divmod(b0 + k, T)
                             nc.tensor.matmul(yp[:, k, :],
                                              res[ci][h][:, j, t, :],
                                              wm16[h], start=False,
                                              stop=(k == bn - 1),
                                              skip_group_check=True)
                         j0 = b0 // T
                         j1 = (b0 + bn - 1) // T + 1
                         dst = st[:, j0:j1, :, h, :]
                         if cp_i % 2 == 0:
                             nc.vector.tensor_copy(out=dst, in_=yp[:, :bn, :])
                         else:
                             nc.scalar.copy(out=dst, in_=yp[:, :bn, :])
                         cp_i += 1
                 nc.scalar.dma_start(out=yv[ci], in_=st)

    nc.compile()
    return nc


def _get_nc(variant=()):
    key = ("nc",) + tuple(sorted(variant))
    if key not in _STATE:
        _STATE[key] = _build_nc(variant)
    return _STATE[key]


def _consts():
    g16 = np.eye(P, dtype=np.float16)
    eye = np.eye(P, dtype=np.float32)
    epseye = (EPS * np.eye(P)).astype(np.float32)
    mask = np.zeros((P, P), dtype=np.float32)
    for g in range(P // 16):
        mask[g * 16:(g + 1) * 16, g * 16:(g + 1) * 16] = 1.0
    bsel = np.zeros((P, 16), dtype=np.float32)
    for k in range(P):
        bsel[k, k % 16] = 1.0
    brep = np.zeros((64, P), dtype=np.float32)
    for j in range(P):
        brep[j % 16, j] = 1.0
        brep[32 + j % 16, j] = 1.0
    ones16 = np.ones((1, P), dtype=np.float16)
    return {"c_id16": g16, "c_eye": eye, "c_epseye": epseye, "c_mask": mask,
            "c_bsel": bsel, "c_brep": brep, "c_ones16": ones16}


def _run(x, trace=False, variant=()):
    from concourse.bass_utils import run_bass_kernel_spmd

    x = np.ascontiguousarray(x, dtype=np.float32).reshape(B, W * H * C)
    consts = _consts()
    in_maps = []
    for i in range(N_CORES):
        m = {"x": np.ascontiguousarray(
            x[i * B_LOC:(i + 1) * B_LOC].reshape(N_LOC, C))}
        m.update(consts)
        in_maps.append(m)

    nc = _get_nc(variant)
    r = run_bass_kernel_spmd(nc, in_maps, core_ids=list(range(N_CORES)),
                             trace=trace)
    out = np.concatenate([r.results[i]["y"].reshape(B_LOC, W, H, C)
                          for i in range(N_CORES)], axis=0)
    return out, r


def kernel(inputs):
    return _run(inputs, trace=False)[0]


if __name__ == "__main__":
    x = np.random.randn(B, W, H, C).astype(np.float32)
    out, _ = _run(x)
    print(out.shape, out.dtype)


# revision 13
# speedup vs baseline: 1.3426x; 1.3426x over previous
"""Decorrelation (ZCA-whitening) normalization kernel for Trainium2 (Bass/Tile).

Full input (64, 56, 56, 256) f32. Data-parallel over batch across 8 NeuronCores
(8 batches -> 25088 pixels per core). Per core:

  Pass 1: stream pixel-major chunks from HBM with 2KB DMA segments (row-pair
          layout: partition p holds DRAM rows 2p,2p+1 of a 256-row block),
          cast to fp16 (gpsimd), accumulate per-half 128x129 Gram blocks on
          the PE (extra ones-column yields channel sums for free), PE-transpose
          every (128px,128ch) tile to channel-major fp16 resident in SBUF
          (copies split across vector/scalar).
  Stats:  compact the block-diagonal Gram to a (34,128) payload (PE select +
          transpose matmuls), one 17KB AllReduce, expand back on each core.
  NS:     trace-normalized Newton-Schulz in Q-form (scale factors folded into
          per-iteration constants): per iter AB = Q@[Q|Sn] (one 256-wide
          matmul), C = A@B, Q' = s_k*C + Q. Both halves interleaved.
  Pass 2: whiten matmul from the fp16 resident tiles (lhsT=res slice,
          rhs=wm16), mean subtraction folded in as a K=1 rank-1 matmul
          accumulating -wm@mu into the same PSUM group, staging copies split
          vector/scalar, 2KB-segment output DMA.

HBM traffic per core = 1x read + 1x write.
"""

import sys

import numpy as np

for _p in ("/root/.axon_site/_ro/trn_rl_repo", "/opt/trn_rl_repo"):
    if _p not in sys.path:
        sys.path.append(_p)

# ---------------------------------------------------------------- constants
B, W, H, C = 64, 56, 56, 256
N_CORES = 8
B_LOC = B // N_CORES                # 8 batches per core
N_LOC = B_LOC * W * H               # 25088 pixels per core
N_TOT = B * W * H                   # 200704 pixels total
P = 128                             # partitions
J = 7                               # row-pair groups per chunk
T = 2                               # rows per pair
CPX = J * T * P                     # 1792 pixels per chunk
NCHUNK = N_LOC // CPX               # 14 chunks per core
EPS = 1e-3
ITER_NUM = 5

assert NCHUNK * CPX == N_LOC

_STATE = {}


def _build_nc(variant=()):
    import concourse.bacc as bacc
    import concourse.tile as tile
    from concourse import mybir
    from contextlib import ExitStack

    f32 = mybir.dt.float32
    f16 = mybir.dt.float16
    Alu = mybir.AluOpType
    Act = mybir.ActivationFunctionType
    Axis = mybir.AxisListType

    nc = bacc.Bacc("TRN2", target_bir_lowering=False, debug=False,
                   num_devices=N_CORES)

    x = nc.dram_tensor("x", [N_LOC, C], f32, kind="ExternalInput").ap()
    y = nc.dram_tensor("y", [N_LOC, C], f32, kind="ExternalOutput").ap()
    c_id16 = nc.dram_tensor("c_id16", [P, P], f16, kind="ExternalInput").ap()
    c_eye = nc.dram_tensor("c_eye", [P, P], f32, kind="ExternalInput").ap()
    c_epseye = nc.dram_tensor("c_epseye", [P, P], f32, kind="ExternalInput").ap()
    c_mask = nc.dram_tensor("c_mask", [P, P], f32, kind="ExternalInput").ap()
    c_bsel = nc.dram_tensor("c_bsel", [P, 16], f32, kind="ExternalInput").ap()
    c_brep = nc.dram_tensor("c_brep", [64, P], f32, kind="ExternalInput").ap()
    c_ones16 = nc.dram_tensor("c_ones16", [1, P], f16, kind="ExternalInput").ap()
    d_ccsb = nc.dram_tensor("d_ccsb", [33, 2, P], f32, kind="ExternalOutput").ap()
    d_arst = nc.dram_tensor("d_arst", [33, 2, P], f32, kind="ExternalOutput").ap()
    d_wm0 = nc.dram_tensor("d_wm0", [P, P], f32, kind="ExternalOutput").ap()
    d_wm1 = nc.dram_tensor("d_wm1", [P, P], f32, kind="ExternalOutput").ap()
    d_nwm = nc.dram_tensor("d_nwm", [2, 4, P], f32, kind="ExternalOutput").ap()

    # Q-form NS constants: Q1 = I - Sn/3; Q_{k+1} = s_k*C_k + Q_k with
    # C_k = Q^3 Sn; final wm = f5 * Q5 * rsqrt(trace).
    fks = [1.5]
    for _ in range(ITER_NUM - 1):
        fks.append(fks[-1] * 1.5)
    sks = [-(fks[k] ** 2) / 3.0 for k in range(ITER_NUM - 1)]
    f_fin = fks[-1]

    with tile.TileContext(nc) as tc, ExitStack() as octx:
        # ---------------- long-lived pools
        consts = octx.enter_context(tc.tile_pool(name="consts", bufs=1))
        resp = octx.enter_context(tc.tile_pool(name="resident", bufs=1))
        statp = octx.enter_context(tc.tile_pool(name="stats", bufs=1))
        gps = octx.enter_context(tc.tile_pool(name="gpsum", bufs=1, space="PSUM"))

        id16 = consts.tile([P, P], f16, name="id16")
        eye = consts.tile([P, P], f32, name="eye")
        epseye = consts.tile([P, P], f32, name="epseye")
        mask = consts.tile([P, P], f32, name="mask")
        bsel = consts.tile([P, 16], f32, name="bsel")
        brep = consts.tile([64, P], f32, name="brep")
        ones16 = consts.tile([1, P], f16, name="ones16")
        nc.sync.dma_start(out=id16, in_=c_id16)
        nc.sync.dma_start(out=eye, in_=c_eye)
        nc.sync.dma_start(out=epseye, in_=c_epseye)
        nc.sync.dma_start(out=mask, in_=c_mask)
        nc.sync.dma_start(out=bsel, in_=c_bsel)
        nc.sync.dma_start(out=brep, in_=c_brep)
        nc.sync.dma_start(out=ones16, in_=c_ones16)

        # channel-major fp16 resident tiles: one per (chunk, half)
        res = [[resp.tile([P, J, T, P], f16, name=f"res_{c}_{h}")
                for h in range(2)] for c in range(NCHUNK)]

        # Gram PSUM: per half (128 x 129), col 128 accumulates channel sums
        g_ps = [gps.tile([P, 129], f32, name=f"G_{h}") for h in range(2)]

        # fp16 cast buffers with baked-in ones columns (manual ping-pong so
        # the ones columns are written exactly once per buffer)
        xh_bufs = [resp.tile([P, J, T, 2, 129], f16, name=f"xh_{k}")
                   for k in range(2)]
        for k in range(2):
            for h in range(2):
                nc.gpsimd.memset(xh_bufs[k][:, :, :, h, 128:129], 1.0)

        xv = x.rearrange("(c j p t) (hh ch) -> c p j t hh ch",
                         j=J, p=P, t=T, hh=2, ch=P)
        yv = y.rearrange("(c j p t) (hh ch) -> c p j t hh ch",
                         j=J, p=P, t=T, hh=2, ch=P)

        nrep = 1
        for v in variant:
            if v.startswith("rep"):
                nrep = int(v[3:])
        for _rep in range(nrep):
         # ================= PASS 1 =================
         with ExitStack() as ctx:
             loadp = ctx.enter_context(tc.tile_pool(name="loadp", bufs=2))
             trps = ctx.enter_context(tc.tile_pool(name="trpsum", bufs=4, space="PSUM"))

             cp_i = 0
             for ci in range(NCHUNK):
                 xt = loadp.tile([P, J, T, 2, P], f32, name="xt")
                 nc.sync.dma_start(out=xt, in_=xv[ci])
                 xh = xh_bufs[ci % 2]
                 # cast f32 -> fp16, one half each on vector/scalar (the
                 # gpsimd Q7 runs this ~4x slower than DVE/ACT)
                 nc.vector.tensor_copy(out=xh[:, :, :, 0, 0:128],
                                       in_=xt[:, :, :, 0, :])
                 nc.scalar.copy(out=xh[:, :, :, 1, 0:128],
                                in_=xt[:, :, :, 1, :])

                 # Gram accumulation (fp16 in, f32 PSUM): G_h += sl^T @ [sl|1]
                 for j in range(J):
                     for t in range(T):
                         first = ci == 0 and j == 0 and t == 0
                         last = ci == NCHUNK - 1 and j == J - 1 and t == T - 1
                         if "nogram" in variant:
                             continue
                         for h in range(2):
                             sl = xh[:, j, t, h, 0:128]
                             rhs = xh[:, j, t, h, 0:129]
                             nc.tensor.matmul(g_ps[h], sl, rhs, start=first,
                                              stop=last, skip_group_check=True)

                 # PE transpose each (128px,128ch) tile -> channel-major fp16
                 for h in range(2 if "notr" not in variant else 0):
                     for b0 in range(0, J * T, 4):
                         bn = min(4, J * T - b0)
                         tp = trps.tile([P, 4, P], f16, name="tp")
                         for k in range(bn):
                             j, t = divmod(b0 + k, T)
                             nc.tensor.matmul(
                                 tp[:, k, :], xh[:, j, t, h, 0:128],
                                 id16, is_transpose=True, skip_group_check=True)
                         j0, t0 = divmod(b0, T)
                         j1 = divmod(b0 + bn - 1, T)[0] + 1
                         dst = res[ci][h][:, j0:j1, :, :]
                         eng = nc.vector if cp_i % 2 == 0 else nc.scalar
                         if eng is nc.vector:
                             nc.vector.tensor_copy(out=dst, in_=tp[:, :bn, :])
                         else:
                             nc.scalar.copy(out=dst, in_=tp[:, :bn, :])
                         cp_i += 1

         # ================= STATS + ALL-REDUCE =================
         with ExitStack() as ctx:
             dramp = ctx.enter_context(tc.tile_pool(name="dram", bufs=1, space="DRAM"))
             nsp = ctx.enter_context(tc.tile_pool(name="nsp", bufs=6))
             nps = ctx.enter_context(tc.tile_pool(name="nspsum", bufs=2, space="PSUM"))

             # masked gram + sums, compacted to a (33, 2, 128) payload:
             # partitions [0:16) = per-half compact gram rows (PE transpose
             # must land at partition 0), partition 32 = channel-sums row
             # (engine partition bases must be 32-aligned)
             Gm = statp.tile([P, 2, 129], f32, name="Gm")
             compactS = statp.tile([P, 2, 17], f32, name="compactS")
             cp_ps = nps.tile([P, 32], f32, name="cp_ps", tag="nsps")
             pay_ps = nps.tile([17, 2, P], f32, name="pay_ps", tag="nsps")
             sums_ps = nps.tile([1, 2, P], f32, name="sums_ps", tag="nsc")
             cc_sb = statp.tile([33, 2, P], f32, name="cc_sb")
             for h in range(2):
                 nc.vector.tensor_tensor(out=Gm[:, h, 0:128],
                                         in0=g_ps[h][:, 0:128], in1=mask,
                                         op=Alu.mult)
                 nc.scalar.copy(out=compactS[:, h, 16:17],
                                in_=g_ps[h][:, 128:129])
                 nc.tensor.matmul(cp_ps[:, h * 16:(h + 1) * 16],
                                  Gm[:, h, 0:128], bsel,
                                  skip_group_check=True)
                 nc.scalar.copy(out=compactS[:, h, 0:16],
                                in_=cp_ps[:, h * 16:(h + 1) * 16])
                 nc.tensor.matmul(pay_ps[0:16, h, :],
                                  compactS[:, h, 0:16], eye,
                                  is_transpose=True, skip_group_check=True)
                 nc.tensor.matmul(sums_ps[0:1, h, :],
                                  compactS[:, h, 16:17], eye,
                                  is_transpose=True, skip_group_check=True)
             nc.scalar.copy(out=cc_sb[0:16, :, :], in_=pay_ps[0:16, :, :])
             nc.scalar.copy(out=cc_sb[32:33, :, :], in_=sums_ps)

             arst = statp.tile([33, 2, P], f32, name="arst")
             if "nocc" in variant:
                 nc.vector.tensor_scalar_mul(out=arst, in0=cc_sb,
                                             scalar1=float(N_CORES))
             else:
                 cc_in = dramp.tile([33, 2, P], f32, name="cc_in")
                 cc_out = dramp.tile([33, 2, P], f32, name="cc_out")
                 nc.sync.dma_start(out=cc_in, in_=cc_sb)
                 nc.gpsimd.collective_compute(
                     "AllReduce", mybir.AluOpType.add,
                     replica_groups=[list(range(N_CORES))],
                     ins=[cc_in.opt()], outs=[cc_out.opt()])
                 nc.sync.dma_start(out=arst, in_=cc_out)

             # ============= Newton-Schulz (halves interleaved) =============
             wm16 = [statp.tile([P, P], f16, name=f"wm16_{h}") for h in range(2)]
             nmu = [statp.tile([P, 1], f32, name=f"nmu_{h}") for h in range(2)]
             nsbuf = [[statp.tile([P, 256], f32, name=f"nsb_{h}_{k}")
                       for k in range(2)] for h in range(2)]

             murow = [arst[32:33, h, :] for h in range(2)]
             sig = [None, None]
             tvec = [None, None]
             rinv = [None, None]

             for h in range(2):
                 # expand compact gram to block-diagonal (128,128)
                 gx_ps = nps.tile([P, P], f32, name=f"gx_{h}", tag="nsps")
                 nc.tensor.matmul(gx_ps, arst[0:16, h, :], brep[0:16, :],
                                  skip_group_check=True)
                 # outer product of channel sums (K=1 matmul)
                 o_ps = nps.tile([P, P], f32, name=f"o_{h}", tag="nsps")
                 nc.tensor.matmul(o_ps, murow[h], murow[h],
                                  skip_group_check=True)
                 osc = nsp.tile([P, P], f32, name="osc", tag="nsbig")
                 nc.scalar.activation(
                     out=osc, in_=o_ps, func=Act.Identity,
                     scale=-(1.0 - EPS) / (float(N_TOT) * float(N_TOT)))
                 # sigma = mask * ((1-eps)/N * G - (1-eps) mu mu^T) + eps I
                 s_t = nsp.tile([P, P], f32, name=f"sig_{h}", tag="sig")
                 nc.vector.scalar_tensor_tensor(
                     out=s_t, in0=gx_ps, scalar=(1.0 - EPS) / float(N_TOT),
                     in1=osc, op0=Alu.mult, op1=Alu.add)
                 nc.vector.tensor_mul(out=s_t, in0=s_t, in1=mask)
                 nc.vector.tensor_add(out=s_t, in0=s_t, in1=epseye)
                 sig[h] = s_t

             for h in range(2):
                 # per-group trace, spread to rows via mask matmul
                 djunk = nsp.tile([P, P], f32, name="djunk", tag="nsbig")
                 dcol = nsp.tile([P, 1], f32, name="dcol", tag="nssmall")
                 nc.vector.tensor_mul(out=djunk, in0=sig[h], in1=eye)
                 nc.vector.reduce_sum(out=dcol, in_=djunk, axis=Axis.X)
                 tv_ps = nps.tile([P, 1], f32, name="tv_ps", tag="nsps")
                 nc.tensor.matmul(tv_ps, mask, dcol, skip_group_check=True)
                 t_t = nsp.tile([P, 1], f32, name=f"tvec_{h}", tag="nssmall")
                 nc.scalar.copy(out=t_t, in_=tv_ps)
                 r_t = nsp.tile([P, 1], f32, name=f"rinv_{h}", tag="nssmall")
                 nc.vector.reciprocal(out=r_t, in_=t_t)
                 tvec[h] = t_t
                 rinv[h] = r_t

             for h in range(2):
                 # Sn into right half of both ping-pong buffers
                 nc.vector.tensor_scalar_mul(out=nsbuf[h][0][:, 128:256],
                                             in0=sig[h], scalar1=rinv[h])
                 nc.scalar.copy(out=nsbuf[h][1][:, 128:256],
                                in_=nsbuf[h][0][:, 128:256])
                 # Q1 = I - Sn/3
                 nc.vector.scalar_tensor_tensor(
                     out=nsbuf[h][0][:, 0:128],
                     in0=nsbuf[h][0][:, 128:256], scalar=-1.0 / 3.0,
                     in1=eye, op0=Alu.mult, op1=Alu.add)

             for k in range(ITER_NUM - 1):
                 src, dst = k % 2, (k + 1) % 2
                 ab_ps = [None, None]
                 absb = [None, None]
                 c_ps = [None, None]
                 for h in range(2):
                     ab_ps[h] = nps.tile([P, 256], f32, name=f"ab_{h}",
                                         tag="nsab")
                     nc.tensor.matmul(ab_ps[h], nsbuf[h][src][:, 0:128],
                                      nsbuf[h][src], skip_group_check=True)
                 for h in range(2):
                     absb[h] = nsp.tile([P, 256], f32, name=f"absb_{h}",
                                        tag=f"nsabsb{h}")
                     nc.scalar.copy(out=absb[h], in_=ab_ps[h])
                 for h in range(2):
                     c_ps[h] = nps.tile([P, P], f32, name=f"c_{h}",
                                        tag="nsc")
                     nc.tensor.matmul(c_ps[h], absb[h][:, 0:128],
                                      absb[h][:, 128:256],
                                      skip_group_check=True)
                 for h in range(2):
                     nc.vector.scalar_tensor_tensor(
                         out=nsbuf[h][dst][:, 0:128], in0=c_ps[h],
                         scalar=sks[k], in1=nsbuf[h][src][:, 0:128],
                         op0=Alu.mult, op1=Alu.add)

             if "dbg" in variant:
                 nc.sync.dma_start(out=d_ccsb, in_=cc_sb)
                 nc.sync.dma_start(out=d_arst, in_=arst)

             qfin = (ITER_NUM - 1) % 2
             for h in range(2):
                 # wm = f_fin * Q5 * rsqrt(trace)
                 sq = nsp.tile([P, 1], f32, name="sq", tag="nssmall")
                 nc.scalar.activation(out=sq, in_=tvec[h], func=Act.Sqrt)
                 rs = nsp.tile([P, 1], f32, name="rs", tag="nssmall")
                 nc.vector.reciprocal(out=rs, in_=sq)
                 wmf = nsp.tile([P, P], f32, name=f"wmf_{h}", tag="wmf")
                 nc.vector.tensor_scalar(out=wmf, in0=nsbuf[h][qfin][:, 0:128],
                                         scalar1=rs, scalar2=float(f_fin),
                                         op0=Alu.mult, op1=Alu.mult)
                 nc.vector.tensor_copy(out=wm16[h], in_=wmf)

                 # -mean column (per-partition bias for the fp16 res tiles)
                 mu_ps = nps.tile([P, 1], f32, name="mu_ps", tag="nsps")
                 nc.tensor.matmul(mu_ps, murow[h], eye[32:33, 32:33],
                                  is_transpose=True, skip_group_check=True)
                 nc.scalar.activation(out=nmu[h], in_=mu_ps, func=Act.Identity,
                                      scale=-1.0 / float(N_TOT))

         if "dbg" in variant:
             dbg_wm = [d_wm0, d_wm1]
             for h in range(2):
                 wmf32 = statp.tile([P, P], f32, name=f"dbgwm_{h}")
                 nc.vector.tensor_copy(out=wmf32, in_=wm16[h])
                 nc.sync.dma_start(out=dbg_wm[h], in_=wmf32)


         # ================= PASS 2 =================
         with ExitStack() as ctx:
             stagep = ctx.enter_context(tc.tile_pool(name="stagep", bufs=2))
             yps = ctx.enter_context(tc.tile_pool(name="ypsum", bufs=6, space="PSUM"))

             cp_i = 0
             for ci in range(NCHUNK if "nop2" not in variant else 0):
                 st = stagep.tile([P, J, T, 2, P], f32, name="st")
                 for h in range(2):
                     # subtract mean in place (per-partition bias, fp16)
                     nc.vector.tensor_scalar_add(out=res[ci][h],
                                                 in0=res[ci][h],
                                                 scalar1=nmu[h])
                     for b0 in range(0, J * T, 4):
                         bn = min(4, J * T - b0)
                         yp = yps.tile([P, 4, P], f32, name="yp")
                         for k in range(bn):
                             j, t = divmod(b0 + k, T)
                             nc.tensor.matmul(yp[:, k, :],
                                              res[ci][h][:, j, t, :],
                                              wm16[h],
                                              skip_group_check=True)
                         j0 = b0 // T
                         j1 = (b0 + bn - 1) // T + 1
                         dst = st[:, j0:j1, :, h, :]
                         # vector also does the mean subtracts; scalar takes
                         # the larger share of the staging copies
                         if cp_i % 8 in (0, 3, 6):
                             nc.vector.tensor_copy(out=dst, in_=yp[:, :bn, :])
                         else:
                             nc.scalar.copy(out=dst, in_=yp[:, :bn, :])
                         cp_i += 1
                 nc.scalar.dma_start(out=yv[ci], in_=st)

    nc.compile()
    return nc


def _get_nc(variant=()):
    key = ("nc",) + tuple(sorted(variant))
    if key not in _STATE:
        _STATE[key] = _build_nc(variant)
    return _STATE[key]


def _consts():
    g16 = np.eye(P, dtype=np.float16)
    eye = np.eye(P, dtype=np.float32)
    epseye = (EPS * np.eye(P)).astype(np.float32)
    mask = np.zeros((P, P), dtype=np.float32)
    for g in range(P // 16):
        mask[g * 16:(g + 1) * 16, g * 16:(g + 1) * 16] = 1.0
    bsel = np.zeros((P, 16), dtype=np.float32)
    for k in range(P):
        bsel[k, k % 16] = 1.0
    brep = np.zeros((64, P), dtype=np.float32)
    for j in range(P):
        brep[j % 16, j] = 1.0
        brep[32 + j % 16, j] = 1.0
    ones16 = np.ones((1, P), dtype=np.float16)
    return {"c_id16": g16, "c_eye": eye, "c_epseye": epseye, "c_mask": mask,
            "c_bsel": bsel, "c_brep": brep, "c_ones16": ones16}


def _run(x, trace=False, variant=()):
    from concourse.bass_utils import run_bass_kernel_spmd

    x = np.ascontiguousarray(x, dtype=np.float32).reshape(B, W * H * C)
    consts = _consts()
    in_maps = []
    for i in range(N_CORES):
        m = {"x": np.ascontiguousarray(
            x[i * B_LOC:(i + 1) * B_LOC].reshape(N_LOC, C))}
        m.update(consts)
        in_maps.append(m)

    nc = _get_nc(variant)
    r = run_bass_kernel_spmd(nc, in_maps, core_ids=list(range(N_CORES)),
                             trace=trace)
    out = np.concatenate([r.results[i]["y"].reshape(B_LOC, W, H, C)
                          for i in range(N_CORES)], axis=0)
    return out, r


def kernel(inputs):
    return _run(inputs, trace=False)[0]


if __name__ == "__main__":
    x = np.random.randn(B, W, H, C).astype(np.float32)
    out, _ = _run(x)
    print(out.shape, out.dtype)


# revision 14
# speedup vs baseline: 1.4040x; 1.0458x over previous
"""Decorrelation (ZCA-whitening) normalization kernel for Trainium2 (Bass/Tile).

Full input (64, 56, 56, 256) f32. Data-parallel over batch across 8 NeuronCores
(8 batches -> 25088 pixels per core). Per core:

  Pass 1: stream pixel-major chunks from HBM with 2KB DMA segments (row-pair
          layout: partition p holds DRAM rows 2p,2p+1 of a 256-row block),
          cast to fp16 (gpsimd), accumulate per-half 128x129 Gram blocks on
          the PE (extra ones-column yields channel sums for free), PE-transpose
          every (128px,128ch) tile to channel-major fp16 resident in SBUF
          (copies split across vector/scalar).
  Stats:  compact the block-diagonal Gram to a (34,128) payload (PE select +
          transpose matmuls), one 17KB AllReduce, expand back on each core.
  NS:     trace-normalized Newton-Schulz in Q-form (scale factors folded into
          per-iteration constants): per iter AB = Q@[Q|Sn] (one 256-wide
          matmul), C = A@B, Q' = s_k*C + Q. Both halves interleaved.
  Pass 2: whiten matmul from the fp16 resident tiles (lhsT=res slice,
          rhs=wm16), mean subtraction folded in as a K=1 rank-1 matmul
          accumulating -wm@mu into the same PSUM group, staging copies split
          vector/scalar, 2KB-segment output DMA.

HBM traffic per core = 1x read + 1x write.
"""

import sys

import numpy as np

for _p in ("/root/.axon_site/_ro/trn_rl_repo", "/opt/trn_rl_repo"):
    if _p not in sys.path:
        sys.path.append(_p)

# ---------------------------------------------------------------- constants
B, W, H, C = 64, 56, 56, 256
N_CORES = 8
B_LOC = B // N_CORES                # 8 batches per core
N_LOC = B_LOC * W * H               # 25088 pixels per core
N_TOT = B * W * H                   # 200704 pixels total
P = 128                             # partitions
J = 7                               # row-pair groups per chunk
T = 2                               # rows per pair
CPX = J * T * P                     # 1792 pixels per chunk
NCHUNK = N_LOC // CPX               # 14 chunks per core
EPS = 1e-3
ITER_NUM = 5

assert NCHUNK * CPX == N_LOC

_STATE = {}


def _build_nc(variant=()):
    import concourse.bacc as bacc
    import concourse.tile as tile
    from concourse import mybir
    from contextlib import ExitStack

    f32 = mybir.dt.float32
    f16 = mybir.dt.float16
    Alu = mybir.AluOpType
    Act = mybir.ActivationFunctionType
    Axis = mybir.AxisListType

    nc = bacc.Bacc("TRN2", target_bir_lowering=False, debug=False,
                   num_devices=N_CORES)

    x = nc.dram_tensor("x", [N_LOC, C], f32, kind="ExternalInput").ap()
    y = nc.dram_tensor("y", [N_LOC, C], f32, kind="ExternalOutput").ap()
    c_id16 = nc.dram_tensor("c_id16", [P, P], f16, kind="ExternalInput").ap()
    c_eye = nc.dram_tensor("c_eye", [P, P], f32, kind="ExternalInput").ap()
    c_epseye = nc.dram_tensor("c_epseye", [P, P], f32, kind="ExternalInput").ap()
    c_mask = nc.dram_tensor("c_mask", [P, P], f32, kind="ExternalInput").ap()
    c_bsel = nc.dram_tensor("c_bsel", [P, 16], f32, kind="ExternalInput").ap()
    c_brep = nc.dram_tensor("c_brep", [64, P], f32, kind="ExternalInput").ap()
    c_ones16 = nc.dram_tensor("c_ones16", [1, P], f16, kind="ExternalInput").ap()
    d_ccsb = nc.dram_tensor("d_ccsb", [33, 2, P], f32, kind="ExternalOutput").ap()
    d_arst = nc.dram_tensor("d_arst", [33, 2, P], f32, kind="ExternalOutput").ap()
    d_wm0 = nc.dram_tensor("d_wm0", [P, P], f32, kind="ExternalOutput").ap()
    d_wm1 = nc.dram_tensor("d_wm1", [P, P], f32, kind="ExternalOutput").ap()
    d_nwm = nc.dram_tensor("d_nwm", [2, 4, P], f32, kind="ExternalOutput").ap()

    # Q-form NS constants: Q1 = I - Sn/3; Q_{k+1} = s_k*C_k + Q_k with
    # C_k = Q^3 Sn; final wm = f5 * Q5 * rsqrt(trace).
    fks = [1.5]
    for _ in range(ITER_NUM - 1):
        fks.append(fks[-1] * 1.5)
    sks = [-(fks[k] ** 2) / 3.0 for k in range(ITER_NUM - 1)]
    f_fin = fks[-1]

    with tile.TileContext(nc) as tc, ExitStack() as octx:
        # ---------------- long-lived pools
        consts = octx.enter_context(tc.tile_pool(name="consts", bufs=1))
        resp = octx.enter_context(tc.tile_pool(name="resident", bufs=1))
        statp = octx.enter_context(tc.tile_pool(name="stats", bufs=1))
        gps = octx.enter_context(tc.tile_pool(name="gpsum", bufs=1, space="PSUM"))

        id16 = consts.tile([P, P], f16, name="id16")
        eye = consts.tile([P, P], f32, name="eye")
        epseye = consts.tile([P, P], f32, name="epseye")
        mask = consts.tile([P, P], f32, name="mask")
        bsel = consts.tile([P, 16], f32, name="bsel")
        brep = consts.tile([64, P], f32, name="brep")
        ones16 = consts.tile([1, P], f16, name="ones16")
        nc.sync.dma_start(out=id16, in_=c_id16)
        nc.sync.dma_start(out=eye, in_=c_eye)
        nc.sync.dma_start(out=epseye, in_=c_epseye)
        nc.sync.dma_start(out=mask, in_=c_mask)
        nc.sync.dma_start(out=bsel, in_=c_bsel)
        nc.sync.dma_start(out=brep, in_=c_brep)
        nc.sync.dma_start(out=ones16, in_=c_ones16)

        # channel-major fp16 resident tiles: one per (chunk, half)
        res = [[resp.tile([P, J, T, P], f16, name=f"res_{c}_{h}")
                for h in range(2)] for c in range(NCHUNK)]

        # Gram PSUM: per half (128 x 129), col 128 accumulates channel sums
        g_ps = [gps.tile([P, 129], f32, name=f"G_{h}") for h in range(2)]

        # fp16 cast buffers with baked-in ones columns (manual ping-pong so
        # the ones columns are written exactly once per buffer)
        xh_bufs = [resp.tile([P, J, T, 2, 129], f16, name=f"xh_{k}")
                   for k in range(2)]
        for k in range(2):
            for h in range(2):
                nc.gpsimd.memset(xh_bufs[k][:, :, :, h, 128:129], 1.0)

        xv = x.rearrange("(c j p t) (hh ch) -> c p j t hh ch",
                         j=J, p=P, t=T, hh=2, ch=P)
        yv = y.rearrange("(c j p t) (hh ch) -> c p j t hh ch",
                         j=J, p=P, t=T, hh=2, ch=P)

        nrep = 1
        for v in variant:
            if v.startswith("rep"):
                nrep = int(v[3:])
        for _rep in range(nrep):
         # ================= PASS 1 =================
         with ExitStack() as ctx:
             loadp = ctx.enter_context(tc.tile_pool(name="loadp", bufs=2))
             trps = ctx.enter_context(tc.tile_pool(name="trpsum", bufs=4, space="PSUM"))

             cp_i = 0
             for ci in range(NCHUNK):
                 xt = loadp.tile([P, J, T, 2, P], f32, name="xt")
                 nc.sync.dma_start(out=xt, in_=xv[ci])
                 xh = xh_bufs[ci % 2]
                 # cast f32 -> fp16, one half each on vector/scalar (the
                 # gpsimd Q7 runs this ~4x slower than DVE/ACT)
                 nc.vector.tensor_copy(out=xh[:, :, :, 0, 0:128],
                                       in_=xt[:, :, :, 0, :])
                 nc.scalar.copy(out=xh[:, :, :, 1, 0:128],
                                in_=xt[:, :, :, 1, :])

                 # Gram accumulation (fp16 in, f32 PSUM): G_h += sl^T @ [sl|1]
                 for j in range(J):
                     for t in range(T):
                         first = ci == 0 and j == 0 and t == 0
                         last = ci == NCHUNK - 1 and j == J - 1 and t == T - 1
                         if "nogram" in variant:
                             continue
                         for h in range(2):
                             sl = xh[:, j, t, h, 0:128]
                             rhs = xh[:, j, t, h, 0:129]
                             nc.tensor.matmul(g_ps[h], sl, rhs, start=first,
                                              stop=last, skip_group_check=True)

                 # PE transpose each (128px,128ch) tile -> channel-major fp16
                 for h in range(2 if "notr" not in variant else 0):
                     for b0 in range(0, J * T, 4):
                         bn = min(4, J * T - b0)
                         tp = trps.tile([P, 4, P], f16, name="tp")
                         for k in range(bn):
                             j, t = divmod(b0 + k, T)
                             nc.tensor.matmul(
                                 tp[:, k, :], xh[:, j, t, h, 0:128],
                                 id16, is_transpose=True, skip_group_check=True)
                         j0, t0 = divmod(b0, T)
                         j1 = divmod(b0 + bn - 1, T)[0] + 1
                         dst = res[ci][h][:, j0:j1, :, :]
                         eng = nc.vector if cp_i % 2 == 0 else nc.scalar
                         if eng is nc.vector:
                             nc.vector.tensor_copy(out=dst, in_=tp[:, :bn, :])
                         else:
                             nc.scalar.copy(out=dst, in_=tp[:, :bn, :])
                         cp_i += 1

         # ================= STATS + ALL-REDUCE =================
         with ExitStack() as ctx:
             dramp = ctx.enter_context(tc.tile_pool(name="dram", bufs=1, space="DRAM"))
             nsp = ctx.enter_context(tc.tile_pool(name="nsp", bufs=6))
             nps = ctx.enter_context(tc.tile_pool(name="nspsum", bufs=2, space="PSUM"))

             # masked gram + sums, compacted to a (33, 2, 128) payload:
             # partitions [0:16) = per-half compact gram rows (PE transpose
             # must land at partition 0), partition 32 = channel-sums row
             # (engine partition bases must be 32-aligned)
             Gm = statp.tile([P, 2, 129], f32, name="Gm")
             compactS = statp.tile([P, 2, 17], f32, name="compactS")
             cp_ps = nps.tile([P, 32], f32, name="cp_ps", tag="nsps")
             pay_ps = nps.tile([17, 2, P], f32, name="pay_ps", tag="nsps")
             sums_ps = nps.tile([1, 2, P], f32, name="sums_ps", tag="nsc")
             cc_sb = statp.tile([33, 2, P], f32, name="cc_sb")
             for h in range(2):
                 nc.vector.tensor_tensor(out=Gm[:, h, 0:128],
                                         in0=g_ps[h][:, 0:128], in1=mask,
                                         op=Alu.mult)
                 nc.scalar.copy(out=compactS[:, h, 16:17],
                                in_=g_ps[h][:, 128:129])
                 nc.tensor.matmul(cp_ps[:, h * 16:(h + 1) * 16],
                                  Gm[:, h, 0:128], bsel,
                                  skip_group_check=True)
                 nc.scalar.copy(out=compactS[:, h, 0:16],
                                in_=cp_ps[:, h * 16:(h + 1) * 16])
                 nc.tensor.matmul(pay_ps[0:16, h, :],
                                  compactS[:, h, 0:16], eye,
                                  is_transpose=True, skip_group_check=True)
                 nc.tensor.matmul(sums_ps[0:1, h, :],
                                  compactS[:, h, 16:17], eye,
                                  is_transpose=True, skip_group_check=True)
             nc.scalar.copy(out=cc_sb[0:16, :, :], in_=pay_ps[0:16, :, :])
             nc.scalar.copy(out=cc_sb[32:33, :, :], in_=sums_ps)

             arst = statp.tile([33, 2, P], f32, name="arst")
             if "nocc" in variant:
                 nc.vector.tensor_scalar_mul(out=arst, in0=cc_sb,
                                             scalar1=float(N_CORES))
             else:
                 cc_in = dramp.tile([33, 2, P], f32, name="cc_in")
                 cc_out = dramp.tile([33, 2, P], f32, name="cc_out")
                 nc.sync.dma_start(out=cc_in, in_=cc_sb)
                 nc.gpsimd.collective_compute(
                     "AllReduce", mybir.AluOpType.add,
                     replica_groups=[list(range(N_CORES))],
                     ins=[cc_in.opt()], outs=[cc_out.opt()])
                 nc.sync.dma_start(out=arst, in_=cc_out)

             # ============= Newton-Schulz (halves interleaved) =============
             wm16 = [statp.tile([P, P], f16, name=f"wm16_{h}") for h in range(2)]
             nmu = [statp.tile([P, 1], f32, name=f"nmu_{h}") for h in range(2)]
             nsbuf = [[statp.tile([P, 256], f32, name=f"nsb_{h}_{k}")
                       for k in range(2)] for h in range(2)]

             murow = [arst[32:33, h, :] for h in range(2)]
             sig = [None, None]
             tvec = [None, None]
             rinv = [None, None]

             for h in range(2):
                 # expand compact gram to block-diagonal (128,128)
                 gx_ps = nps.tile([P, P], f32, name=f"gx_{h}", tag="nsps")
                 nc.tensor.matmul(gx_ps, arst[0:16, h, :], brep[0:16, :],
                                  skip_group_check=True)
                 # outer product of channel sums (K=1 matmul)
                 o_ps = nps.tile([P, P], f32, name=f"o_{h}", tag="nsps")
                 nc.tensor.matmul(o_ps, murow[h], murow[h],
                                  skip_group_check=True)
                 osc = nsp.tile([P, P], f32, name="osc", tag="nsbig")
                 nc.scalar.activation(
                     out=osc, in_=o_ps, func=Act.Identity,
                     scale=-(1.0 - EPS) / (float(N_TOT) * float(N_TOT)))
                 # sigma = mask * ((1-eps)/N * G - (1-eps) mu mu^T) + eps I
                 s_t = nsp.tile([P, P], f32, name=f"sig_{h}", tag="sig")
                 nc.vector.scalar_tensor_tensor(
                     out=s_t, in0=gx_ps, scalar=(1.0 - EPS) / float(N_TOT),
                     in1=osc, op0=Alu.mult, op1=Alu.add)
                 nc.vector.tensor_mul(out=s_t, in0=s_t, in1=mask)
                 nc.vector.tensor_add(out=s_t, in0=s_t, in1=epseye)
                 sig[h] = s_t

             for h in range(2):
                 # per-group trace, spread to rows via mask matmul
                 djunk = nsp.tile([P, P], f32, name="djunk", tag="nsbig")
                 dcol = nsp.tile([P, 1], f32, name="dcol", tag="nssmall")
                 nc.vector.tensor_mul(out=djunk, in0=sig[h], in1=eye)
                 nc.vector.reduce_sum(out=dcol, in_=djunk, axis=Axis.X)
                 tv_ps = nps.tile([P, 1], f32, name="tv_ps", tag="nsps")
                 nc.tensor.matmul(tv_ps, mask, dcol, skip_group_check=True)
                 t_t = nsp.tile([P, 1], f32, name=f"tvec_{h}", tag="nssmall")
                 nc.scalar.copy(out=t_t, in_=tv_ps)
                 r_t = nsp.tile([P, 1], f32, name=f"rinv_{h}", tag="nssmall")
                 nc.vector.reciprocal(out=r_t, in_=t_t)
                 tvec[h] = t_t
                 rinv[h] = r_t

             for h in range(2):
                 # Sn into right half of both ping-pong buffers
                 nc.vector.tensor_scalar_mul(out=nsbuf[h][0][:, 128:256],
                                             in0=sig[h], scalar1=rinv[h])
                 nc.scalar.copy(out=nsbuf[h][1][:, 128:256],
                                in_=nsbuf[h][0][:, 128:256])
                 # Q1 = I - Sn/3
                 nc.vector.scalar_tensor_tensor(
                     out=nsbuf[h][0][:, 0:128],
                     in0=nsbuf[h][0][:, 128:256], scalar=-1.0 / 3.0,
                     in1=eye, op0=Alu.mult, op1=Alu.add)

             for k in range(ITER_NUM - 1):
                 src, dst = k % 2, (k + 1) % 2
                 ab_ps = [None, None]
                 absb = [None, None]
                 c_ps = [None, None]
                 for h in range(2):
                     ab_ps[h] = nps.tile([P, 256], f32, name=f"ab_{h}",
                                         tag="nsab")
                     nc.tensor.matmul(ab_ps[h], nsbuf[h][src][:, 0:128],
                                      nsbuf[h][src], skip_group_check=True)
                 for h in range(2):
                     absb[h] = nsp.tile([P, 256], f32, name=f"absb_{h}",
                                        tag=f"nsabsb{h}")
                     nc.scalar.copy(out=absb[h], in_=ab_ps[h])
                 for h in range(2):
                     c_ps[h] = nps.tile([P, P], f32, name=f"c_{h}",
                                        tag="nsc")
                     nc.tensor.matmul(c_ps[h], absb[h][:, 0:128],
                                      absb[h][:, 128:256],
                                      skip_group_check=True)
                 for h in range(2):
                     nc.vector.scalar_tensor_tensor(
                         out=nsbuf[h][dst][:, 0:128], in0=c_ps[h],
                         scalar=sks[k], in1=nsbuf[h][src][:, 0:128],
                         op0=Alu.mult, op1=Alu.add)

             if "dbg" in variant:
                 nc.sync.dma_start(out=d_ccsb, in_=cc_sb)
                 nc.sync.dma_start(out=d_arst, in_=arst)

             qfin = (ITER_NUM - 1) % 2
             for h in range(2):
                 # wm = f_fin * Q5 * rsqrt(trace)
                 sq = nsp.tile([P, 1], f32, name="sq", tag="nssmall")
                 nc.scalar.activation(out=sq, in_=tvec[h], func=Act.Sqrt)
                 rs = nsp.tile([P, 1], f32, name="rs", tag="nssmall")
                 nc.vector.reciprocal(out=rs, in_=sq)
                 wmf = nsp.tile([P, P], f32, name=f"wmf_{h}", tag="wmf")
                 nc.vector.tensor_scalar(out=wmf, in0=nsbuf[h][qfin][:, 0:128],
                                         scalar1=rs, scalar2=float(f_fin),
                                         op0=Alu.mult, op1=Alu.mult)
                 nc.vector.tensor_copy(out=wm16[h], in_=wmf)

                 # -mean column (per-partition bias for the fp16 res tiles)
                 mu_ps = nps.tile([P, 1], f32, name="mu_ps", tag="nsps")
                 nc.tensor.matmul(mu_ps, murow[h], eye[32:33, 32:33],
                                  is_transpose=True, skip_group_check=True)
                 nc.scalar.activation(out=nmu[h], in_=mu_ps, func=Act.Identity,
                                      scale=-1.0 / float(N_TOT))

         if "dbg" in variant:
             dbg_wm = [d_wm0, d_wm1]
             for h in range(2):
                 wmf32 = statp.tile([P, P], f32, name=f"dbgwm_{h}")
                 nc.vector.tensor_copy(out=wmf32, in_=wm16[h])
                 nc.sync.dma_start(out=dbg_wm[h], in_=wmf32)


         # ================= PASS 2 =================
         with ExitStack() as ctx:
             stagep = ctx.enter_context(tc.tile_pool(name="stagep", bufs=2))
             yps = ctx.enter_context(tc.tile_pool(name="ypsum", bufs=6, space="PSUM"))

             cp_i = 0
             for ci in range(NCHUNK if "nop2" not in variant else 0):
                 st = stagep.tile([P, J, T, 2, P], f32, name="st")
                 for h in range(2):
                     # subtract mean in place (per-partition bias, fp16)
                     nc.vector.tensor_scalar_add(out=res[ci][h],
                                                 in0=res[ci][h],
                                                 scalar1=nmu[h])
                     for b0 in range(0, J * T, 4):
                         bn = min(4, J * T - b0)
                         yp = yps.tile([P, 4, P], f32, name="yp")
                         for k in range(bn):
                             j, t = divmod(b0 + k, T)
                             nc.tensor.matmul(yp[:, k, :],
                                              res[ci][h][:, j, t, :],
                                              wm16[h],
                                              skip_group_check=True)
                         j0 = b0 // T
                         j1 = (b0 + bn - 1) // T + 1
                         dst = st[:, j0:j1, :, h, :]
                         # all staging copies on scalar: vector's queue holds
                         # only the mean subtracts, which then all run ahead
                         # of the PE instead of lockstepping with it
                         nc.scalar.copy(out=dst, in_=yp[:, :bn, :])
                         cp_i += 1
                 nc.sync.dma_start(out=yv[ci], in_=st)

    nc.compile()
    return nc


def _get_nc(variant=()):
    key = ("nc",) + tuple(sorted(variant))
    if key not in _STATE:
        _STATE[key] = _build_nc(variant)
    return _STATE[key]


def _consts():
    g16 = np.eye(P, dtype=np.float16)
    eye = np.eye(P, dtype=np.float32)
    epseye = (EPS * np.eye(P)).astype(np.float32)
    mask = np.zeros((P, P), dtype=np.float32)
    for g in range(P // 16):
        mask[g * 16:(g + 1) * 16, g * 16:(g + 1) * 16] = 1.0
    bsel = np.zeros((P, 16), dtype=np.float32)
    for k in range(P):
        bsel[k, k % 16] = 1.0
    brep = np.zeros((64, P), dtype=np.float32)
    for j in range(P):
        brep[j % 16, j] = 1.0
        brep[32 + j % 16, j] = 1.0
    ones16 = np.ones((1, P), dtype=np.float16)
    return {"c_id16": g16, "c_eye": eye, "c_epseye": epseye, "c_mask": mask,
            "c_bsel": bsel, "c_brep": brep, "c_ones16": ones16}


def _run(x, trace=False, variant=()):
    from concourse.bass_utils import run_bass_kernel_spmd

    x = np.ascontiguousarray(x, dtype=np.float32).reshape(B, W * H * C)
    consts = _consts()
    in_maps = []
    for i in range(N_CORES):
        m = {"x": np.ascontiguousarray(
            x[i * B_LOC:(i + 1) * B_LOC].reshape(N_LOC, C))}
        m.update(consts)
        in_maps.append(m)

    nc = _get_nc(variant)
    r = run_bass_kernel_spmd(nc, in_maps, core_ids=list(range(N_CORES)),
                             trace=trace)
    out = np.concatenate([r.results[i]["y"].reshape(B_LOC, W, H, C)
                          for i in range(N_CORES)], axis=0)
    return out, r


def kernel(inputs):
    return _run(inputs, trace=False)[0]


if __name__ == "__main__":
    x = np.random.randn(B, W, H, C).astype(np.float32)
    out, _ = _run(x)
    print(out.shape, out.dtype)
